# revision 3
# baseline (speedup 1.0000x reference)
"""AugGraphConv (per-relation GAT + lang-level softmax) on 8 TRN2 NeuronCores.

v3 strategy (dst-sharded graph parallel; minimize host->device bytes AND
instruction count -- this runtime costs ~70us per instruction):
  - Core m receives ONLY its x shard [S=6272, D] bf16; full x assembled on
    device via AllGather. Small params shipped as single rows and expanded
    on device by outer-product matmuls.
  - Stage A: LN batched 4 tiles per instruction group; per-relation
    feat_r = xn @ [W_r | u_r] written to DRAM [NPAD, R*FD] bf16, one DMA per
    4 tiles. ar (att_dst logits) -> DRAM arrel [S, R*H]; self path + x kept
    SBUF-resident.
  - Stage B: edges binned by (own dst tile, relation), chunks of 128 edges,
    elementwise work batched over 4 chunks; num|den fused into ONE scatter
    matmul per chunk via a [V | w] rhs. Per-edge ar gathered from arrel with
    a per-chunk element offset (same dsti index stream as the one-hot).
    Padded edges: dsti=200 -> one-hot column zero; arrel padded+zeroed so
    t=48 reads stay finite.
  - Epilogue + lang softmax batched across the 5 relations per tile; W_cross
    applied once per tile via softmax linearity: sum_k w_k (g_k @ Wc) =
    (sum_k w_k g_k) @ Wc; per-relation lang logits via g_r @ cw with
    cw = blockdiag(att_src_lang) folded into W_cross on the host.
  - Direct DMAs ride the two HWDGE queues (sync/scalar engines); the 4 SWDGE
    queues are reserved for indirect gathers (round-robin).
"""

import os
import numpy as np
import ml_dtypes
from contextlib import ExitStack

import concourse.bass as bass
import concourse.mybir as mybir
from concourse.bass import IndirectOffsetOnAxis
from concourse.tile import TileContext
from concourse.bass_utils import run_bass_kernel_spmd

N, D, H, R, C = 50000, 128, 8, 5, 16
P = 128
M = 8
NPAD = 50176            # 392 * 128, divisible by M*P
S = NPAD // M           # 6272 rows per core
T = S // P              # 49 owned tiles per core
GT = NPAD // P          # 392 global tiles
FD = D + H              # 136: [xw | al]
RH = R * H              # 40
PADR = 256              # arrel pad rows (dsti=200 + t*128 stays in bounds)
F32 = mybir.dt.float32
BF16 = mybir.dt.bfloat16
I32 = mybir.dt.int32
AF = mybir.ActivationFunctionType
ALU = mybir.AluOpType
AX = mybir.AxisListType
NEGM = -30.0            # softmax mask value (exp(-30) ~ 1e-13, negligible)
NSWQ = int(os.environ.get("NSWQ", "4"))  # SWDGE queues for gathers
BQ = 4                  # chunk batch (elementwise ops grouped over BQ chunks)
BT = 4                  # stage-A tile batch

LAST_RESULTS = None     # test.py reads exec_time_ns / profile from here


def _split_multiwaits(nc):
    """This toolchain's walrus codegen allows only one sem-wait per
    instruction; hoist extra waits into preceding NoOps on the same engine
    (sequencer executes them in program order, so semantics are identical)."""
    n_split = 0
    for _, bbwrap in nc.bb_map.items():
        bb = bbwrap.bb
        out = []
        changed = False
        for inst in list(bb.instructions):
            si = inst.sync_info
            if si is not None and si.on_wait is not None and len(si.on_wait) > 1:
                waits = list(si.on_wait)
                for w in waits[:-1]:
                    out.append(mybir.InstNoOp(
                        name=nc.get_next_instruction_name(),
                        engine=inst.engine, ins=[], outs=[],
                        sync_info=mybir.SyncInfo(on_wait=[w], on_update=[])))
                    n_split += 1
                si.on_wait = waits[-1:]
                inst.sync_info = si
                changed = True
            out.append(inst)
        if changed:
            bb.instructions = out
    return n_split


def _spread_indirect_queues(nc, nq):
    """Spread SWDGE DMAs over nq queues BY COMPLETION SEMAPHORE: Tile tracks
    each DMA on a per-DMA semaphore drawn from a rotating pool (DMASW<j>_*)
    with cumulative wait thresholds on reuse, so all DMAs sharing semaphore j
    must stay FIFO-ordered -- which holds iff they share one queue. Mapping
    queue = j % nq preserves per-sem ordering while using all queues."""
    import re
    n = 0
    for _, bbwrap in nc.bb_map.items():
        for inst in bbwrap.bb.instructions:
            if isinstance(inst, mybir.InstDMACopy) and inst.queue == "qPoolDynamic":
                si = inst.sync_info
                j = None
                for u in (si.on_update or []) if si else []:
                    m = re.match(r"DMASW(\d+)_", u.ant_name or "")
                    if m:
                        j = int(m.group(1))
                if j is not None:
                    inst.queue = f"qPoolDynamic{j % nq or ''}"
                    n += 1
    return n


def _build(K, TOTC):
    nc = bass.Bass(num_devices=M, num_swdge_queues=NSWQ)
    x_shard = nc.declare_dram_parameter("x_shard", [S, D], BF16, isOutput=False)
    srcg = nc.declare_dram_parameter("src_gidx", [P, TOTC], I32, isOutput=False)
    dsti = nc.declare_dram_parameter("dst_idx", [P, TOTC], I32, isOutput=False)
    maskq = nc.declare_dram_parameter("maskq", [P, T * R], BF16, isOutput=False)
    wcat = nc.declare_dram_parameter("wcat", [D, R * FD], BF16, isOutput=False)
    wown = nc.declare_dram_parameter("wown", [D, RH + D], BF16, isOutput=False)
    wcross = nc.declare_dram_parameter("wcross", [D, D], BF16, isOutput=False)
    cw = nc.declare_dram_parameter("cw", [D, H], BF16, isOutput=False)
    prow = nc.declare_dram_parameter("prow", [1, 8 * D], F32, isOutput=False)
    iota = nc.declare_dram_parameter("iota_f", [P, P], BF16, isOutput=False)
    idenb = nc.declare_dram_parameter("ident_b", [P, P], BF16, isOutput=False)
    out = nc.declare_dram_parameter("out", [S, D], BF16, isOutput=True)

    featw = nc.dram_tensor("feat_all", [NPAD, R * FD], BF16)
    arrel = nc.dram_tensor("ar_rel", [S + PADR, RH], BF16)

    with TileContext(nc) as tc, ExitStack() as ctx:
        cp = ctx.enter_context(tc.tile_pool(name="const", bufs=1))
        sb = ctx.enter_context(tc.tile_pool(name="sb", bufs=3))
        eb = ctx.enter_context(tc.tile_pool(name="eb", bufs=8))
        lb = ctx.enter_context(tc.tile_pool(name="lb", bufs=2))
        psA = ctx.enter_context(tc.tile_pool(name="psA", bufs=2, space="PSUM"))
        psF = ctx.enter_context(tc.tile_pool(name="psF", bufs=1, space="PSUM"))
        psB = ctx.enter_context(tc.tile_pool(name="psB", bufs=2, space="PSUM"))
        psL = ctx.enter_context(tc.tile_pool(name="psL", bufs=1, space="PSUM"))
        dram = ctx.enter_context(tc.tile_pool(name="dram", bufs=1, space="DRAM"))

        # ---- persistent constants / index arrays ----
        wcat_s = cp.tile([D, R * FD], BF16)
        nc.sync.dma_start(out=wcat_s[:], in_=wcat[:])
        wown_s = cp.tile([D, RH + D], BF16)
        nc.sync.dma_start(out=wown_s[:], in_=wown[:])
        wcross_s = cp.tile([D, D], BF16)
        nc.sync.dma_start(out=wcross_s[:], in_=wcross[:])
        cw_s = cp.tile([D, H], BF16)
        nc.sync.dma_start(out=cw_s[:], in_=cw[:])
        iota_s = cp.tile([P, P], BF16)
        nc.sync.dma_start(out=iota_s[:], in_=iota[:])
        idenb_s = cp.tile([P, P], BF16)
        nc.sync.dma_start(out=idenb_s[:], in_=idenb[:])
        srcg_s = cp.tile([P, TOTC], I32)
        nc.sync.dma_start(out=srcg_s[:], in_=srcg[:])
        dsti_s = cp.tile([P, TOTC], I32)
        nc.sync.dma_start(out=dsti_s[:], in_=dsti[:])
        maskq_s = cp.tile([P, T * R], BF16)
        nc.sync.dma_start(out=maskq_s[:], in_=maskq[:])
        dstl_s = cp.tile([P, TOTC], BF16)
        nc.vector.tensor_copy(out=dstl_s[:], in_=dsti_s[:])
        # expand single-row params to [P, *] via outer products with ones
        prow_s = cp.tile([1, 8 * D], F32)
        nc.sync.dma_start(out=prow_s[:], in_=prow[:])
        ones1 = cp.tile([1, P], F32)
        nc.vector.memset(ones1[:], 1.0)
        asl_s = cp.tile([P, D], F32)
        adl_s = cp.tile([P, D], F32)
        bl_s = cp.tile([P, D], F32)
        pp = psL.tile([P, D], F32, tag="vv")
        nc.tensor.matmul(out=pp[:], lhsT=ones1[:], rhs=prow_s[:, 0:D],
                         start=True, stop=True)
        nc.vector.tensor_copy(out=asl_s[:], in_=pp[:])
        pp = psL.tile([P, D], F32, tag="vv")
        nc.tensor.matmul(out=pp[:], lhsT=ones1[:], rhs=prow_s[:, D:2 * D],
                         start=True, stop=True)
        nc.vector.tensor_copy(out=adl_s[:], in_=pp[:])
        pp = psL.tile([P, D], F32, tag="vv")
        nc.tensor.matmul(out=pp[:], lhsT=ones1[:], rhs=prow_s[:, 2 * D:3 * D],
                         start=True, stop=True)
        nc.vector.tensor_copy(out=bl_s[:], in_=pp[:])
        bw_s = cp.tile([P, R * D], F32)
        pf = psF.tile([P, 3 * FD], F32, tag="fmA")
        nc.tensor.matmul(out=pf[:, 0:3 * FD], lhsT=ones1[:],
                         rhs=prow_s[:, 3 * D:3 * D + 3 * FD], start=True, stop=True)
        nc.vector.tensor_copy(out=bw_s[:, 0:3 * FD], in_=pf[:])
        pf = psF.tile([P, 2 * FD], F32, tag="fmB")
        nc.tensor.matmul(out=pf[:, 0:R * D - 3 * FD], lhsT=ones1[:],
                         rhs=prow_s[:, 3 * D + 3 * FD:3 * D + R * D],
                         start=True, stop=True)
        nc.vector.tensor_copy(out=bw_s[:, 3 * FD:R * D],
                              in_=pf[:, 0:R * D - 3 * FD])
        # SBUF-resident per-core state
        xsh_all = cp.tile([P, T * D], BF16)    # owned x tiles (residual)
        sown_all = cp.tile([P, T * D], F32)    # self-path rows

        # zero arrel pad rows (pad dsti=200 for t=48 lands in [S, S+PADR))
        zpad = cp.tile([P, RH], BF16)
        nc.vector.memset(zpad[:], 0.0)
        nc.scalar.dma_start(out=arrel[S:S + P, :], in_=zpad[:])
        nc.scalar.dma_start(out=arrel[S + P:S + PADR, :], in_=zpad[:])

        # ---- assemble full x on device ----
        xb = dram.tile([S, D], BF16)
        xg = dram.tile([NPAD, D], BF16)
        nc.scalar.dma_start(out=xb[:], in_=x_shard[:])
        nc.gpsimd.collective_compute(
            "AllGather", ALU.bypass,
            replica_groups=[list(range(M))],
            ins=[xb[:].opt()], outs=[xg[:].opt()])

        def ln_batch(xv, bsz):
            """LayerNorm bsz tiles from a [P, bsz, D] bf16 view; returns
            bf16 [P, bsz, D] normalized tile."""
            mu = sb.tile([P, BT], F32, tag="mu")
            nc.vector.tensor_reduce(out=mu[:, 0:bsz], in_=xv, axis=AX.X,
                                    op=ALU.add)
            nc.vector.tensor_scalar_mul(out=mu[:, 0:bsz], in0=mu[:, 0:bsz],
                                        scalar1=1.0 / D)
            xc = sb.tile([P, BT, D], F32, tag="xc")
            nc.vector.tensor_tensor(
                out=xc[:, 0:bsz, :], in0=xv,
                in1=mu[:, 0:bsz, None].to_broadcast([P, bsz, D]),
                op=ALU.subtract)
            sq = sb.tile([P, BT, D], F32, tag="sq")
            nc.scalar.activation(out=sq[:, 0:bsz, :], in_=xc[:, 0:bsz, :],
                                 func=AF.Square)
            var = sb.tile([P, BT], F32, tag="var")
            nc.vector.tensor_reduce(out=var[:, 0:bsz], in_=sq[:, 0:bsz, :],
                                    axis=AX.X, op=ALU.add)
            nc.vector.tensor_scalar(out=var[:, 0:bsz], in0=var[:, 0:bsz],
                                    scalar1=1.0 / D, scalar2=1e-5,
                                    op0=ALU.mult, op1=ALU.add)
            sd = sb.tile([P, BT], F32, tag="sd")
            nc.scalar.activation(out=sd[:, 0:bsz], in_=var[:, 0:bsz],
                                 func=AF.Sqrt)
            rs = sb.tile([P, BT], F32, tag="rs")
            nc.vector.reciprocal(out=rs[:, 0:bsz], in_=sd[:, 0:bsz])
            xn = sb.tile([P, BT, D], BF16, tag="xn")
            nc.vector.tensor_tensor(
                out=xn[:, 0:bsz, :], in0=xc[:, 0:bsz, :],
                in1=rs[:, 0:bsz, None].to_broadcast([P, bsz, D]),
                op=ALU.mult)
            return xn

        def transpose_tiles(xn, bsz):
            """PE-transpose bsz [P, D] bf16 slices; returns [P, bsz, P] bf16."""
            xnT = sb.tile([P, BT, P], BF16, tag="xnT")
            for b in range(bsz):
                tp = psA.tile([P, P], BF16, tag="tp")
                nc.tensor.transpose(out=tp[:], in_=xn[:, b, :],
                                    identity=idenb_s[:])
                nc.vector.tensor_copy(out=xnT[:, b, :], in_=tp[:])
            return xnT

        # ---- Stage A (owned): ar + self path from the local shard ----
        for t0 in range(0, T, BT):
            bsz = min(BT, T - t0)
            nc.sync.dma_start(
                out=xsh_all[:, t0 * D:(t0 + bsz) * D].rearrange(
                    "p (b d) -> p b d", d=D),
                in_=x_shard[t0 * P:(t0 + bsz) * P, :].rearrange(
                    "(b p) d -> p b d", p=P))
            xv = xsh_all[:, t0 * D:(t0 + bsz) * D].rearrange(
                "p (b d) -> p b d", d=D)
            xn = ln_batch(xv, bsz)
            xnT = transpose_tiles(xn, bsz)
            for b in range(bsz):
                po = psF.tile([P, 3 * FD], F32, tag="fmA")
                nc.tensor.matmul(out=po[:, 0:RH + D], lhsT=xnT[:, b, :],
                                 rhs=wown_s[:], start=True, stop=True)
                ac = sb.tile([P, RH], BF16, tag="ac")
                nc.vector.tensor_copy(out=ac[:], in_=po[:, 0:RH])
                nc.scalar.dma_start(
                    out=arrel[(t0 + b) * P:(t0 + b + 1) * P, :], in_=ac[:])
                nc.vector.tensor_copy(
                    out=sown_all[:, (t0 + b) * D:(t0 + b + 1) * D],
                    in_=po[:, RH:RH + D])

        # ---- Stage A (global): per-relation features for all nodes ----
        for g0 in range(0, GT, BT):
            bsz = min(BT, GT - g0)
            xt = sb.tile([P, BT, D], BF16, tag="xt")
            nc.sync.dma_start(
                out=xt[:, 0:bsz, :],
                in_=xg[g0 * P:(g0 + bsz) * P, :].rearrange(
                    "(b p) d -> p b d", p=P))
            xn = ln_batch(xt[:, 0:bsz, :], bsz)
            xnT = transpose_tiles(xn, bsz)
            fc = sb.tile([P, BT, R * FD], BF16, tag="fc")
            for b in range(bsz):
                fmA = psF.tile([P, 3 * FD], F32, tag="fmA")
                nc.tensor.matmul(out=fmA[:], lhsT=xnT[:, b, :],
                                 rhs=wcat_s[:, 0:3 * FD], start=True, stop=True)
                nc.vector.tensor_copy(out=fc[:, b, 0:3 * FD], in_=fmA[:])
                fmB = psF.tile([P, 2 * FD], F32, tag="fmB")
                nc.tensor.matmul(out=fmB[:], lhsT=xnT[:, b, :],
                                 rhs=wcat_s[:, 3 * FD:5 * FD], start=True, stop=True)
                nc.vector.tensor_copy(out=fc[:, b, 3 * FD:5 * FD], in_=fmB[:])
            nc.scalar.dma_start(
                out=featw[g0 * P:(g0 + bsz) * P, :].rearrange(
                    "(b p) f -> p b f", p=P),
                in_=fc[:, 0:bsz, :])

        # ---- Stage B: edge aggregation + lang softmax, per owned tile ----
        c = 0
        for t in range(T):
            nd_all = lb.tile([P, R, FD], F32, tag="nd_all")
            for r in range(R):
                Kt = K[t][r]
                numden = psB.tile([P, FD], F32, tag="nd")
                k = 0
                for k0 in range(0, Kt, BQ):
                    B = min(BQ, Kt - k0)
                    G4 = eb.tile([P, BQ, FD], BF16, tag="G")
                    Aar4 = eb.tile([P, BQ, H], BF16, tag="Aar")
                    for b in range(B):
                        nc.gpsimd.indirect_dma_start(
                            out=G4[:, b, :], out_offset=None, in_=featw[:],
                            in_offset=IndirectOffsetOnAxis(
                                ap=srcg_s[:, c + b:c + b + 1], axis=0),
                            element_offset=r * FD)
                        nc.gpsimd.indirect_dma_start(
                            out=Aar4[:, b, :], out_offset=None, in_=arrel[:],
                            in_offset=IndirectOffsetOnAxis(
                                ap=dsti_s[:, c + b:c + b + 1], axis=0),
                            element_offset=t * P * RH + r * H)
                    Sm4 = eb.tile([P, BQ, P], BF16, tag="Sm")
                    nc.vector.tensor_tensor(
                        out=Sm4[:, 0:B, :],
                        in0=dstl_s[:, c:c + B][:, :, None].to_broadcast([P, B, P]),
                        in1=iota_s[:][:, None, :].to_broadcast([P, B, P]),
                        op=ALU.is_equal)
                    lg4 = eb.tile([P, BQ, H], F32, tag="lg")
                    nc.vector.tensor_add(out=lg4[:, 0:B, :],
                                         in0=G4[:, 0:B, D:FD], in1=Aar4[:, 0:B, :])
                    l24 = eb.tile([P, BQ, H], F32, tag="l2")
                    nc.vector.tensor_scalar_mul(out=l24[:, 0:B, :],
                                                in0=lg4[:, 0:B, :], scalar1=0.2)
                    lr4 = eb.tile([P, BQ, H], F32, tag="lr")
                    nc.vector.tensor_tensor(out=lr4[:, 0:B, :], in0=lg4[:, 0:B, :],
                                            in1=l24[:, 0:B, :], op=ALU.max)
                    w4 = eb.tile([P, BQ, H], F32, tag="w")
                    nc.scalar.activation(out=w4[:, 0:B, :], in_=lr4[:, 0:B, :],
                                         func=AF.Exp)
                    V4 = eb.tile([P, BQ, FD], BF16, tag="V")
                    nc.vector.tensor_copy(out=V4[:, 0:B, D:FD], in_=w4[:, 0:B, :])
                    nc.vector.tensor_tensor(
                        out=V4[:, 0:B, 0:D].rearrange("p b (h c) -> p b h c", c=C),
                        in0=G4[:, 0:B, 0:D].rearrange("p b (h c) -> p b h c", c=C),
                        in1=w4[:, 0:B, :, None].to_broadcast([P, B, H, C]),
                        op=ALU.mult)
                    for b in range(B):
                        nc.tensor.matmul(out=numden[:], lhsT=Sm4[:, b, :],
                                         rhs=V4[:, b, :],
                                         start=(k == 0), stop=(k == Kt - 1))
                        k += 1
                    c += B
                nc.vector.tensor_copy(out=nd_all[:, r, :], in_=numden[:])

            # ---- batched epilogue over the 5 relations ----
            den1 = lb.tile([P, R, H], F32, tag="den1")
            nc.vector.tensor_scalar_max(out=den1[:], in0=nd_all[:, :, D:FD],
                                        scalar1=1e-6)
            rec = lb.tile([P, R, H], F32, tag="rec")
            nc.vector.reciprocal(out=rec[:], in_=den1[:])
            Oall = lb.tile([P, R * D], F32, tag="Oall")
            nc.vector.tensor_tensor(
                out=Oall[:].rearrange("p (r h c) -> p r h c", r=R, c=C),
                in0=nd_all[:, :, 0:D].rearrange("p r (h c) -> p r h c", c=C),
                in1=rec[:, :, :, None].to_broadcast([P, R, H, C]),
                op=ALU.mult)
            nc.vector.tensor_add(out=Oall[:], in0=Oall[:], in1=bw_s[:])
            gb = lb.tile([P, R * D], BF16, tag="gb")
            nc.scalar.activation(out=gb[:], in_=Oall[:], func=AF.Gelu)

            # lang logits: al_r = g_r @ cw for r>=1; al_0, ar from self path
            gT = lb.tile([P, R, P], BF16, tag="gT")
            for r in range(R):
                tp = psA.tile([P, P], BF16, tag="tp")
                nc.tensor.transpose(out=tp[:], in_=gb[:, r * D:(r + 1) * D],
                                    identity=idenb_s[:])
                nc.vector.tensor_copy(out=gT[:, r, :], in_=tp[:])
            alp_ps = psL.tile([P, (R + 1) * H], F32, tag="alp")
            for r in range(R):
                nc.tensor.matmul(out=alp_ps[:, (r + 1) * H:(r + 2) * H],
                                 lhsT=gT[:, r, :], rhs=cw_s[:],
                                 start=True, stop=True)
            alp = lb.tile([P, (R + 1) * H], F32, tag="alp_s")
            nc.vector.tensor_copy(out=alp[:, H:(R + 1) * H],
                                  in_=alp_ps[:, H:(R + 1) * H])
            v0 = sown_all[:, t * D:(t + 1) * D]
            tmp = lb.tile([P, D], F32, tag="ltmp")
            nc.vector.tensor_tensor(out=tmp[:], in0=v0, in1=asl_s[:],
                                    op=ALU.mult)
            nc.vector.tensor_reduce(
                out=alp[:, 0:H], in_=tmp[:].rearrange("p (h c) -> p h c", c=C),
                axis=AX.X, op=ALU.add)
            arl = lb.tile([P, H], F32, tag="arl")
            nc.vector.tensor_tensor(out=tmp[:], in0=v0, in1=adl_s[:],
                                    op=ALU.mult)
            nc.vector.tensor_reduce(
                out=arl[:], in_=tmp[:].rearrange("p (h c) -> p h c", c=C),
                axis=AX.X, op=ALU.add)
            lgp = lb.tile([P, (R + 1) * H], F32, tag="lgp")
            nc.vector.tensor_tensor(
                out=lgp[:].rearrange("p (k h) -> p k h", h=H),
                in0=alp[:].rearrange("p (k h) -> p k h", h=H),
                in1=arl[:, None, :].to_broadcast([P, R + 1, H]),
                op=ALU.add)
            l2p = lb.tile([P, (R + 1) * H], F32, tag="l2p")
            nc.vector.tensor_scalar_mul(out=l2p[:], in0=lgp[:], scalar1=0.2)
            nc.vector.tensor_tensor(out=lgp[:], in0=lgp[:], in1=l2p[:],
                                    op=ALU.max)
            # mask: host-computed bin-occupancy, replicated over heads
            maskp = lb.tile([P, (R + 1) * H], F32, tag="maskp")
            nc.vector.memset(maskp[:, 0:H], 1.0)
            nc.vector.tensor_copy(
                out=maskp[:, H:(R + 1) * H].rearrange("p (r h) -> p r h", h=H),
                in_=maskq_s[:, t * R:(t + 1) * R][:, :, None].to_broadcast(
                    [P, R, H]))
            lm = lb.tile([P, (R + 1) * H], F32, tag="lm")
            nc.vector.tensor_tensor(out=lm[:], in0=lgp[:], in1=maskp[:],
                                    op=ALU.mult)
            mneg = lb.tile([P, (R + 1) * H], F32, tag="mneg")
            nc.vector.tensor_scalar(out=mneg[:], in0=maskp[:], scalar1=1.0,
                                    scalar2=-NEGM, op0=ALU.subtract,
                                    op1=ALU.mult)
            nc.vector.tensor_add(out=lm[:], in0=lm[:], in1=mneg[:])
            ep = lb.tile([P, (R + 1) * H], F32, tag="ep")
            nc.scalar.activation(out=ep[:], in_=lm[:], func=AF.Exp)
            dl = lb.tile([P, H], F32, tag="dl")
            nc.vector.tensor_reduce(
                out=dl[:], in_=ep[:].rearrange("p (k h) -> p h k", h=H),
                axis=AX.X, op=ALU.add)
            rl = lb.tile([P, H], F32, tag="rl")
            nc.vector.reciprocal(out=rl[:], in_=dl[:])
            wga = lb.tile([P, (R + 1) * H], F32, tag="wga")
            nc.vector.tensor_tensor(
                out=wga[:].rearrange("p (k h) -> p k h", h=H),
                in0=ep[:].rearrange("p (k h) -> p k h", h=H),
                in1=rl[:, None, :].to_broadcast([P, R + 1, H]),
                op=ALU.mult)
            # v_r = g_r @ W_cross (reusing gT), then per-head weighted sum
            acc = lb.tile([P, D], F32, tag="acc")
            nc.vector.tensor_tensor(
                out=acc[:].rearrange("p (h c) -> p h c", c=C),
                in0=v0.rearrange("p (h c) -> p h c", c=C),
                in1=wga[:, 0:H][:, :, None].to_broadcast([P, H, C]),
                op=ALU.mult)
            t2 = lb.tile([P, D], F32, tag="t2")
            for r in range(R):
                vv = psL.tile([P, D], F32, tag="vv")
                nc.tensor.matmul(out=vv[:], lhsT=gT[:, r, :], rhs=wcross_s[:],
                                 start=True, stop=True)
                nc.vector.tensor_tensor(
                    out=t2[:].rearrange("p (h c) -> p h c", c=C),
                    in0=vv[:].rearrange("p (h c) -> p h c", c=C),
                    in1=wga[:, (r + 1) * H:(r + 2) * H][:, :, None].to_broadcast(
                        [P, H, C]),
                    op=ALU.mult)
                nc.vector.tensor_add(out=acc[:], in0=acc[:], in1=t2[:])
            nc.vector.tensor_add(out=acc[:], in0=acc[:], in1=bl_s[:])
            go = lb.tile([P, D], F32, tag="go")
            nc.scalar.activation(out=go[:], in_=acc[:], func=AF.Gelu)
            nc.vector.tensor_add(out=go[:], in0=go[:],
                                 in1=xsh_all[:, t * D:(t + 1) * D])
            gob = lb.tile([P, D], BF16, tag="gob")
            nc.vector.tensor_copy(out=gob[:], in_=go[:])
            nc.sync.dma_start(out=out[t * P:(t + 1) * P, :], in_=gob[:])
    return nc


def _prep(x_inp, edge_index, edge_type, W_self, W_word, att_src_word,
          att_dst_word, bias_word, W_cross, att_src_lang, att_dst_lang,
          bias_lang):
    xpad = np.zeros((NPAD, D), np.float32)
    xpad[:N] = x_inp.astype(np.float32)
    src_all = edge_index[0].astype(np.int64)
    dst_all = edge_index[1].astype(np.int64)
    et_all = edge_type.astype(np.int64)

    # shared params
    Wcat = np.zeros((D, R * FD), np.float32)
    Vcat = np.zeros((D, RH), np.float32)
    for r in range(R):
        Wr = W_word[r].astype(np.float32)               # [D, D]
        u = np.einsum('dhc,hc->dh', Wr.reshape(D, H, C),
                      att_src_word[r].astype(np.float32))
        v = np.einsum('dhc,hc->dh', Wr.reshape(D, H, C),
                      att_dst_word[r].astype(np.float32))
        Wcat[:, r * FD:r * FD + D] = Wr
        Wcat[:, r * FD + D:(r + 1) * FD] = u
        Vcat[:, r * H:(r + 1) * H] = v
    Wown = np.concatenate([Vcat, W_self.astype(np.float32)], axis=1)
    Wc = W_cross.astype(np.float32)
    asl = att_src_lang.astype(np.float32)               # [H, C]
    cwm = np.einsum('dhc,hc->dh', Wc.reshape(D, H, C), asl)  # [D, H]
    prow = np.concatenate([
        asl.reshape(D), att_dst_lang.astype(np.float32).reshape(D),
        bias_lang.astype(np.float32).reshape(D),
        bias_word.astype(np.float32).reshape(R * D)]).reshape(1, 8 * D)
    params = {
        "wcat": Wcat.astype(ml_dtypes.bfloat16),
        "wown": Wown.astype(ml_dtypes.bfloat16),
        "wcross": Wc.astype(ml_dtypes.bfloat16),
        "cw": cwm.astype(ml_dtypes.bfloat16),
        "prow": prow,
        "iota_f": np.tile(np.arange(P, dtype=np.float32)[None, :], (P, 1)).astype(ml_dtypes.bfloat16),
        "ident_b": np.eye(P, dtype=np.float32).astype(ml_dtypes.bfloat16),
    }

    # per-core edge binning (dst ownership, global src ids)
    core_of = dst_all // S
    percore = []
    cnts = np.zeros((M, T, R), np.int64)
    for m in range(M):
        sel = core_of == m
        srcm, dstm, etm = src_all[sel], dst_all[sel], et_all[sel]
        dst_l = dstm - m * S
        t_loc = dst_l // P
        order = np.lexsort((dst_l % P, etm, t_loc))
        src_g, dst_l, etm, t_loc = (srcm[order], dst_l[order], etm[order],
                                    t_loc[order])
        cnts[m] = np.bincount(t_loc * R + etm, minlength=T * R).reshape(T, R)
        percore.append((src_g, dst_l, etm, t_loc))

    K = np.maximum(1, -(-cnts.max(axis=0) // P))        # [T, R] chunk counts
    TOTC = int(K.sum())
    coff = np.zeros((T, R), np.int64)                    # chunk offsets
    coff.flat[1:] = np.cumsum(K.flat)[:-1]

    in_maps = []
    for m in range(M):
        src_g, dst_l, etm, t_loc = percore[m]
        sg = np.zeros(TOTC * P, np.int32)
        di = np.full(TOTC * P, 200, np.int32)
        eoff = np.zeros((T, R), np.int64)
        eoff.flat[1:] = np.cumsum(cnts[m].flat)[:-1]
        for t in range(T):
            for r in range(R):
                n_e = cnts[m, t, r]
                if n_e == 0:
                    continue
                o = eoff[t, r]
                slot = coff[t, r] * P + np.arange(n_e)
                sg[slot] = src_g[o:o + n_e]
                di[slot] = dst_l[o:o + n_e] % P
        occ = np.zeros((S, R), np.float32)               # per-node occupancy
        occ[dst_l, etm] = 1.0
        maskm = np.ascontiguousarray(
            occ.reshape(T, P, R).transpose(1, 0, 2).reshape(P, T * R))
        in_maps.append({
            "x_shard": xpad[m * S:(m + 1) * S].astype(ml_dtypes.bfloat16),
            "src_gidx": np.ascontiguousarray(sg.reshape(TOTC, P).T),
            "dst_idx": np.ascontiguousarray(di.reshape(TOTC, P).T),
            "maskq": maskm.astype(ml_dtypes.bfloat16),
            **params,
        })
    return K.tolist(), TOTC, in_maps


def kernel(x_inp, node_type, edge_index, edge_type, W_self, W_word,
           att_src_word, att_dst_word, bias_word, W_cross,
           att_src_lang, att_dst_lang, bias_lang):
    global LAST_RESULTS
    K, TOTC, in_maps = _prep(
        np.asarray(x_inp), np.asarray(edge_index), np.asarray(edge_type),
        np.asarray(W_self), np.asarray(W_word), np.asarray(att_src_word),
        np.asarray(att_dst_word), np.asarray(bias_word), np.asarray(W_cross),
        np.asarray(att_src_lang), np.asarray(att_dst_lang),
        np.asarray(bias_lang))
    nc = _build(K, TOTC)
    _split_multiwaits(nc)
    if NSWQ > 1:
        _spread_indirect_queues(nc, NSWQ)
    global LAST_NC, LAST_INMAPS
    LAST_NC, LAST_INMAPS = nc, in_maps
    res = run_bass_kernel_spmd(nc, in_maps, list(range(M)),
                               trace=bool(os.environ.get("BASS_TRACE")))
    LAST_RESULTS = res
    out = np.concatenate([res.results[m]["out"].astype(np.float32)
                          for m in range(M)], axis=0)
    return out[:N]


# revision 4
# speedup vs baseline: 1.0065x; 1.0065x over previous
"""AugGraphConv (per-relation GAT + lang-level softmax) on 8 TRN2 NeuronCores.

v3 strategy (dst-sharded graph parallel; minimize host->device bytes AND
instruction count -- this runtime costs ~70us per instruction):
  - Core m receives ONLY its x shard [S=6272, D] bf16; full x assembled on
    device via AllGather. Small params shipped as single rows and expanded
    on device by outer-product matmuls.
  - Stage A: LN batched 4 tiles per instruction group; per-relation
    feat_r = xn @ [W_r | u_r] written to DRAM [NPAD, R*FD] bf16, one DMA per
    4 tiles. ar (att_dst logits) -> DRAM arrel [S, R*H]; self path + x kept
    SBUF-resident.
  - Stage B: edges binned by (own dst tile, relation), chunks of 128 edges,
    elementwise work batched over 4 chunks; num|den fused into ONE scatter
    matmul per chunk via a [V | w] rhs. Per-edge ar gathered from arrel with
    a per-chunk element offset (same dsti index stream as the one-hot).
    Padded edges: dsti=200 -> one-hot column zero; arrel padded+zeroed so
    t=48 reads stay finite.
  - Epilogue + lang softmax batched across the 5 relations per tile; W_cross
    applied once per tile via softmax linearity: sum_k w_k (g_k @ Wc) =
    (sum_k w_k g_k) @ Wc; per-relation lang logits via g_r @ cw with
    cw = blockdiag(att_src_lang) folded into W_cross on the host.
  - Direct DMAs ride the two HWDGE queues (sync/scalar engines); the 4 SWDGE
    queues are reserved for indirect gathers (round-robin).
"""

import os
import numpy as np
import ml_dtypes
from contextlib import ExitStack

import concourse.bass as bass
import concourse.mybir as mybir
from concourse.bass import IndirectOffsetOnAxis
from concourse.tile import TileContext
from concourse.bass_utils import run_bass_kernel_spmd

N, D, H, R, C = 50000, 128, 8, 5, 16
P = 128
M = 8
NPAD = 50176            # 392 * 128, divisible by M*P
S = NPAD // M           # 6272 rows per core
T = S // P              # 49 owned tiles per core
GT = NPAD // P          # 392 global tiles
FD = D + H              # 136: [xw | al]
RH = R * H              # 40
PADR = 256              # arrel pad rows (dsti=200 + t*128 stays in bounds)
F32 = mybir.dt.float32
BF16 = mybir.dt.bfloat16
I32 = mybir.dt.int32
AF = mybir.ActivationFunctionType
ALU = mybir.AluOpType
AX = mybir.AxisListType
NEGM = -30.0            # softmax mask value (exp(-30) ~ 1e-13, negligible)
NSWQ = int(os.environ.get("NSWQ", "4"))  # SWDGE queues for gathers
BQ = 4                  # chunk batch (elementwise ops grouped over BQ chunks)
BT = 8                  # stage-A tile batch (max group size)

LAST_RESULTS = None     # test.py reads exec_time_ns / profile from here


def _split_multiwaits(nc):
    """This toolchain's walrus codegen allows only one sem-wait per
    instruction; hoist extra waits into preceding NoOps on the same engine
    (sequencer executes them in program order, so semantics are identical)."""
    n_split = 0
    for _, bbwrap in nc.bb_map.items():
        bb = bbwrap.bb
        out = []
        changed = False
        for inst in list(bb.instructions):
            si = inst.sync_info
            if si is not None and si.on_wait is not None and len(si.on_wait) > 1:
                waits = list(si.on_wait)
                for w in waits[:-1]:
                    out.append(mybir.InstNoOp(
                        name=nc.get_next_instruction_name(),
                        engine=inst.engine, ins=[], outs=[],
                        sync_info=mybir.SyncInfo(on_wait=[w], on_update=[])))
                    n_split += 1
                si.on_wait = waits[-1:]
                inst.sync_info = si
                changed = True
            out.append(inst)
        if changed:
            bb.instructions = out
    return n_split


def _spread_indirect_queues(nc, nq):
    """Spread SWDGE DMAs over nq queues BY COMPLETION SEMAPHORE: Tile tracks
    each DMA on a per-DMA semaphore drawn from a rotating pool (DMASW<j>_*)
    with cumulative wait thresholds on reuse, so all DMAs sharing semaphore j
    must stay FIFO-ordered -- which holds iff they share one queue. Mapping
    queue = j % nq preserves per-sem ordering while using all queues."""
    import re
    n = 0
    for _, bbwrap in nc.bb_map.items():
        for inst in bbwrap.bb.instructions:
            if isinstance(inst, mybir.InstDMACopy) and inst.queue == "qPoolDynamic":
                si = inst.sync_info
                j = None
                for u in (si.on_update or []) if si else []:
                    m = re.match(r"DMASW(\d+)_", u.ant_name or "")
                    if m:
                        j = int(m.group(1))
                if j is not None:
                    inst.queue = f"qPoolDynamic{j % nq or ''}"
                    n += 1
    return n


def _build(K, TOTC):
    nc = bass.Bass(num_devices=M, num_swdge_queues=NSWQ)
    x_shard = nc.declare_dram_parameter("x_shard", [S, D], BF16, isOutput=False)
    eidx = nc.declare_dram_parameter("eidx", [P, TOTC], I32, isOutput=False)
    maskq = nc.declare_dram_parameter("maskq", [P, T * R], BF16, isOutput=False)
    wcat = nc.declare_dram_parameter("wcat", [D, R * FD], BF16, isOutput=False)
    wown = nc.declare_dram_parameter("wown", [D, RH + D], BF16, isOutput=False)
    wcross = nc.declare_dram_parameter("wcross", [D, D], BF16, isOutput=False)
    cw = nc.declare_dram_parameter("cw", [D, H], BF16, isOutput=False)
    prow = nc.declare_dram_parameter("prow", [1, 8 * D], F32, isOutput=False)
    iota = nc.declare_dram_parameter("iota_f", [P, P], BF16, isOutput=False)
    idenb = nc.declare_dram_parameter("ident_b", [P, P], BF16, isOutput=False)
    out = nc.declare_dram_parameter("out", [S, D], BF16, isOutput=True)

    featw = nc.dram_tensor("feat_all", [NPAD, R * FD], BF16)
    arrel = nc.dram_tensor("ar_rel", [S + PADR, RH], BF16)

    with TileContext(nc) as tc, ExitStack() as ctx:
        cp = ctx.enter_context(tc.tile_pool(name="const", bufs=1))
        sb = ctx.enter_context(tc.tile_pool(name="sb", bufs=3))
        eb = ctx.enter_context(tc.tile_pool(name="eb", bufs=8))
        lb = ctx.enter_context(tc.tile_pool(name="lb", bufs=2))
        psA = ctx.enter_context(tc.tile_pool(name="psA", bufs=2, space="PSUM"))
        psF = ctx.enter_context(tc.tile_pool(name="psF", bufs=1, space="PSUM"))
        psB = ctx.enter_context(tc.tile_pool(name="psB", bufs=2, space="PSUM"))
        psL = ctx.enter_context(tc.tile_pool(name="psL", bufs=1, space="PSUM"))
        dram = ctx.enter_context(tc.tile_pool(name="dram", bufs=1, space="DRAM"))

        # ---- persistent constants / index arrays ----
        wcat_s = cp.tile([D, R * FD], BF16)
        nc.sync.dma_start(out=wcat_s[:], in_=wcat[:])
        wown_s = cp.tile([D, RH + D], BF16)
        nc.sync.dma_start(out=wown_s[:], in_=wown[:])
        wcross_s = cp.tile([D, D], BF16)
        nc.sync.dma_start(out=wcross_s[:], in_=wcross[:])
        cw_s = cp.tile([D, H], BF16)
        nc.sync.dma_start(out=cw_s[:], in_=cw[:])
        iota_s = cp.tile([P, P], BF16)
        nc.sync.dma_start(out=iota_s[:], in_=iota[:])
        idenb_s = cp.tile([P, P], BF16)
        nc.sync.dma_start(out=idenb_s[:], in_=idenb[:])
        eidx_s = cp.tile([P, TOTC], I32)
        nc.sync.dma_start(out=eidx_s[:], in_=eidx[:])
        srcg_s = cp.tile([P, TOTC], I32)
        nc.vector.tensor_scalar(out=srcg_s[:], in0=eidx_s[:], scalar1=0xFFFF,
                                scalar2=None, op0=ALU.bitwise_and)
        dsti_s = cp.tile([P, TOTC], I32)
        nc.vector.tensor_scalar(out=dsti_s[:], in0=eidx_s[:], scalar1=16,
                                scalar2=None, op0=ALU.logical_shift_right)
        maskq_s = cp.tile([P, T * R], BF16)
        nc.sync.dma_start(out=maskq_s[:], in_=maskq[:])
        dstl_s = cp.tile([P, TOTC], BF16)
        nc.vector.tensor_copy(out=dstl_s[:], in_=dsti_s[:])
        # expand single-row params to [P, *] via outer products with ones
        prow_s = cp.tile([1, 8 * D], F32)
        nc.sync.dma_start(out=prow_s[:], in_=prow[:])
        ones1 = cp.tile([1, P], F32)
        nc.vector.memset(ones1[:], 1.0)
        asl_s = cp.tile([P, D], F32)
        adl_s = cp.tile([P, D], F32)
        bl_s = cp.tile([P, D], F32)
        pp = psL.tile([P, D], F32, tag="vv")
        nc.tensor.matmul(out=pp[:], lhsT=ones1[:], rhs=prow_s[:, 0:D],
                         start=True, stop=True)
        nc.vector.tensor_copy(out=asl_s[:], in_=pp[:])
        pp = psL.tile([P, D], F32, tag="vv")
        nc.tensor.matmul(out=pp[:], lhsT=ones1[:], rhs=prow_s[:, D:2 * D],
                         start=True, stop=True)
        nc.vector.tensor_copy(out=adl_s[:], in_=pp[:])
        pp = psL.tile([P, D], F32, tag="vv")
        nc.tensor.matmul(out=pp[:], lhsT=ones1[:], rhs=prow_s[:, 2 * D:3 * D],
                         start=True, stop=True)
        nc.vector.tensor_copy(out=bl_s[:], in_=pp[:])
        bw_s = cp.tile([P, R * D], F32)
        pf = psF.tile([P, 3 * FD], F32, tag="fmA")
        nc.tensor.matmul(out=pf[:, 0:3 * FD], lhsT=ones1[:],
                         rhs=prow_s[:, 3 * D:3 * D + 3 * FD], start=True, stop=True)
        nc.vector.tensor_copy(out=bw_s[:, 0:3 * FD], in_=pf[:])
        pf = psF.tile([P, 2 * FD], F32, tag="fmB")
        nc.tensor.matmul(out=pf[:, 0:R * D - 3 * FD], lhsT=ones1[:],
                         rhs=prow_s[:, 3 * D + 3 * FD:3 * D + R * D],
                         start=True, stop=True)
        nc.vector.tensor_copy(out=bw_s[:, 3 * FD:R * D],
                              in_=pf[:, 0:R * D - 3 * FD])
        # SBUF-resident per-core state
        xsh_all = cp.tile([P, T * D], BF16)    # owned x tiles (residual)
        sown_all = cp.tile([P, T * D], F32)    # self-path rows

        # zero arrel pad rows (pad dsti=200 for t=48 lands in [S, S+PADR))
        zpad = cp.tile([P, RH], BF16)
        nc.vector.memset(zpad[:], 0.0)
        nc.scalar.dma_start(out=arrel[S:S + P, :], in_=zpad[:])
        nc.scalar.dma_start(out=arrel[S + P:S + PADR, :], in_=zpad[:])

        # ---- assemble full x on device ----
        xb = dram.tile([S, D], BF16)
        xg = dram.tile([NPAD, D], BF16)
        nc.scalar.dma_start(out=xb[:], in_=x_shard[:])
        nc.gpsimd.collective_compute(
            "AllGather", ALU.bypass,
            replica_groups=[list(range(M))],
            ins=[xb[:].opt()], outs=[xg[:].opt()])

        def ln_batch(xv, bsz):
            """LayerNorm bsz tiles from a [P, bsz, D] bf16 view; returns
            bf16 [P, bsz, D] normalized tile."""
            mu = sb.tile([P, BT], F32, tag="mu")
            nc.vector.tensor_reduce(out=mu[:, 0:bsz], in_=xv, axis=AX.X,
                                    op=ALU.add)
            nc.vector.tensor_scalar_mul(out=mu[:, 0:bsz], in0=mu[:, 0:bsz],
                                        scalar1=1.0 / D)
            xc = sb.tile([P, BT, D], F32, tag="xc")
            nc.vector.tensor_tensor(
                out=xc[:, 0:bsz, :], in0=xv,
                in1=mu[:, 0:bsz, None].to_broadcast([P, bsz, D]),
                op=ALU.subtract)
            sq = sb.tile([P, BT, D], F32, tag="sq")
            nc.scalar.activation(out=sq[:, 0:bsz, :], in_=xc[:, 0:bsz, :],
                                 func=AF.Square)
            var = sb.tile([P, BT], F32, tag="var")
            nc.vector.tensor_reduce(out=var[:, 0:bsz], in_=sq[:, 0:bsz, :],
                                    axis=AX.X, op=ALU.add)
            nc.vector.tensor_scalar(out=var[:, 0:bsz], in0=var[:, 0:bsz],
                                    scalar1=1.0 / D, scalar2=1e-5,
                                    op0=ALU.mult, op1=ALU.add)
            sd = sb.tile([P, BT], F32, tag="sd")
            nc.scalar.activation(out=sd[:, 0:bsz], in_=var[:, 0:bsz],
                                 func=AF.Sqrt)
            rs = sb.tile([P, BT], F32, tag="rs")
            nc.vector.reciprocal(out=rs[:, 0:bsz], in_=sd[:, 0:bsz])
            xn = sb.tile([P, BT, D], BF16, tag="xn")
            nc.vector.tensor_tensor(
                out=xn[:, 0:bsz, :], in0=xc[:, 0:bsz, :],
                in1=rs[:, 0:bsz, None].to_broadcast([P, bsz, D]),
                op=ALU.mult)
            return xn

        def transpose_tiles(xn, bsz):
            """PE-transpose bsz [P, D] bf16 slices; returns [P, bsz, P] bf16."""
            xnT = sb.tile([P, BT, P], BF16, tag="xnT")
            for b in range(bsz):
                tp = psA.tile([P, P], BF16, tag="tp")
                nc.tensor.transpose(out=tp[:], in_=xn[:, b, :],
                                    identity=idenb_s[:])
                nc.vector.tensor_copy(out=xnT[:, b, :], in_=tp[:])
            return xnT

        # ---- Stage A (owned): ar + self path from the local shard ----
        for t0 in range(0, T, 7):
            bsz = min(7, T - t0)
            nc.sync.dma_start(
                out=xsh_all[:, t0 * D:(t0 + bsz) * D].rearrange(
                    "p (b d) -> p b d", d=D),
                in_=x_shard[t0 * P:(t0 + bsz) * P, :].rearrange(
                    "(b p) d -> p b d", p=P))
            xv = xsh_all[:, t0 * D:(t0 + bsz) * D].rearrange(
                "p (b d) -> p b d", d=D)
            xn = ln_batch(xv, bsz)
            xnT = transpose_tiles(xn, bsz)
            for b in range(bsz):
                po = psF.tile([P, 3 * FD], F32, tag="fmA")
                nc.tensor.matmul(out=po[:, 0:RH + D], lhsT=xnT[:, b, :],
                                 rhs=wown_s[:], start=True, stop=True)
                ac = sb.tile([P, RH], BF16, tag="ac")
                nc.vector.tensor_copy(out=ac[:], in_=po[:, 0:RH])
                nc.scalar.dma_start(
                    out=arrel[(t0 + b) * P:(t0 + b + 1) * P, :], in_=ac[:])
                nc.vector.tensor_copy(
                    out=sown_all[:, (t0 + b) * D:(t0 + b + 1) * D],
                    in_=po[:, RH:RH + D])

        # ---- Stage A (global): per-relation features for all nodes ----
        for g0 in range(0, GT, BT):
            bsz = min(BT, GT - g0)
            xt = sb.tile([P, BT, D], BF16, tag="xt")
            nc.sync.dma_start(
                out=xt[:, 0:bsz, :],
                in_=xg[g0 * P:(g0 + bsz) * P, :].rearrange(
                    "(b p) d -> p b d", p=P))
            xn = ln_batch(xt[:, 0:bsz, :], bsz)
            xnT = transpose_tiles(xn, bsz)
            fc = sb.tile([P, BT, R * FD], BF16, tag="fc")
            for b in range(bsz):
                fmA = psF.tile([P, 3 * FD], F32, tag="fmA")
                nc.tensor.matmul(out=fmA[:], lhsT=xnT[:, b, :],
                                 rhs=wcat_s[:, 0:3 * FD], start=True, stop=True)
                nc.vector.tensor_copy(out=fc[:, b, 0:3 * FD], in_=fmA[:])
                fmB = psF.tile([P, 2 * FD], F32, tag="fmB")
                nc.tensor.matmul(out=fmB[:], lhsT=xnT[:, b, :],
                                 rhs=wcat_s[:, 3 * FD:5 * FD], start=True, stop=True)
                nc.vector.tensor_copy(out=fc[:, b, 3 * FD:5 * FD], in_=fmB[:])
            nc.scalar.dma_start(
                out=featw[g0 * P:(g0 + bsz) * P, :].rearrange(
                    "(b p) f -> p b f", p=P),
                in_=fc[:, 0:bsz, :])

        # ---- Stage B: edge aggregation + lang softmax, per owned tile ----
        c = 0
        for t in range(T):
            nd_all = lb.tile([P, R, FD], F32, tag="nd_all")
            for r in range(R):
                Kt = K[t][r]
                numden = psB.tile([P, FD], F32, tag="nd")
                k = 0
                for k0 in range(0, Kt, BQ):
                    B = min(BQ, Kt - k0)
                    G4 = eb.tile([P, BQ, FD], BF16, tag="G")
                    Aar4 = eb.tile([P, BQ, H], BF16, tag="Aar")
                    for b in range(B):
                        nc.gpsimd.indirect_dma_start(
                            out=G4[:, b, :], out_offset=None, in_=featw[:],
                            in_offset=IndirectOffsetOnAxis(
                                ap=srcg_s[:, c + b:c + b + 1], axis=0),
                            element_offset=r * FD)
                        nc.gpsimd.indirect_dma_start(
                            out=Aar4[:, b, :], out_offset=None, in_=arrel[:],
                            in_offset=IndirectOffsetOnAxis(
                                ap=dsti_s[:, c + b:c + b + 1], axis=0),
                            element_offset=t * P * RH + r * H)
                    Sm4 = eb.tile([P, BQ, P], BF16, tag="Sm")
                    nc.vector.tensor_tensor(
                        out=Sm4[:, 0:B, :],
                        in0=dstl_s[:, c:c + B][:, :, None].to_broadcast([P, B, P]),
                        in1=iota_s[:][:, None, :].to_broadcast([P, B, P]),
                        op=ALU.is_equal)
                    lg4 = eb.tile([P, BQ, H], F32, tag="lg")
                    nc.vector.tensor_add(out=lg4[:, 0:B, :],
                                         in0=G4[:, 0:B, D:FD], in1=Aar4[:, 0:B, :])
                    l24 = eb.tile([P, BQ, H], F32, tag="l2")
                    nc.vector.tensor_scalar_mul(out=l24[:, 0:B, :],
                                                in0=lg4[:, 0:B, :], scalar1=0.2)
                    lr4 = eb.tile([P, BQ, H], F32, tag="lr")
                    nc.vector.tensor_tensor(out=lr4[:, 0:B, :], in0=lg4[:, 0:B, :],
                                            in1=l24[:, 0:B, :], op=ALU.max)
                    V4 = eb.tile([P, BQ, FD], BF16, tag="V")
                    nc.scalar.activation(out=V4[:, 0:B, D:FD], in_=lr4[:, 0:B, :],
                                         func=AF.Exp)
                    nc.vector.tensor_tensor(
                        out=V4[:, 0:B, 0:D].rearrange("p b (h c) -> p b h c", c=C),
                        in0=G4[:, 0:B, 0:D].rearrange("p b (h c) -> p b h c", c=C),
                        in1=V4[:, 0:B, D:FD, None].to_broadcast([P, B, H, C]),
                        op=ALU.mult)
                    for b in range(B):
                        nc.tensor.matmul(out=numden[:], lhsT=Sm4[:, b, :],
                                         rhs=V4[:, b, :],
                                         start=(k == 0), stop=(k == Kt - 1))
                        k += 1
                    c += B
                nc.vector.tensor_copy(out=nd_all[:, r, :], in_=numden[:])

            # ---- batched epilogue over the 5 relations ----
            den1 = lb.tile([P, R, H], F32, tag="den1")
            nc.vector.tensor_scalar_max(out=den1[:], in0=nd_all[:, :, D:FD],
                                        scalar1=1e-6)
            rec = lb.tile([P, R, H], F32, tag="rec")
            nc.vector.reciprocal(out=rec[:], in_=den1[:])
            Oall = lb.tile([P, R * D], F32, tag="Oall")
            nc.vector.tensor_tensor(
                out=Oall[:].rearrange("p (r h c) -> p r h c", r=R, c=C),
                in0=nd_all[:, :, 0:D].rearrange("p r (h c) -> p r h c", c=C),
                in1=rec[:, :, :, None].to_broadcast([P, R, H, C]),
                op=ALU.mult)
            nc.vector.tensor_add(out=Oall[:], in0=Oall[:], in1=bw_s[:])
            gb = lb.tile([P, R * D], BF16, tag="gb")
            nc.scalar.activation(out=gb[:], in_=Oall[:], func=AF.Gelu)

            # lang logits: al_r = g_r @ cw for r>=1; al_0, ar from self path
            gT = lb.tile([P, R, P], BF16, tag="gT")
            for r in range(R):
                tp = psA.tile([P, P], BF16, tag="tp")
                nc.tensor.transpose(out=tp[:], in_=gb[:, r * D:(r + 1) * D],
                                    identity=idenb_s[:])
                nc.vector.tensor_copy(out=gT[:, r, :], in_=tp[:])
            alp_ps = psL.tile([P, (R + 1) * H], F32, tag="alp")
            for r in range(R):
                nc.tensor.matmul(out=alp_ps[:, (r + 1) * H:(r + 2) * H],
                                 lhsT=gT[:, r, :], rhs=cw_s[:],
                                 start=True, stop=True)
            alp = lb.tile([P, (R + 1) * H], F32, tag="alp_s")
            nc.vector.tensor_copy(out=alp[:, H:(R + 1) * H],
                                  in_=alp_ps[:, H:(R + 1) * H])
            v0 = sown_all[:, t * D:(t + 1) * D]
            tmp = lb.tile([P, D], F32, tag="ltmp")
            nc.vector.tensor_tensor(out=tmp[:], in0=v0, in1=asl_s[:],
                                    op=ALU.mult)
            nc.vector.tensor_reduce(
                out=alp[:, 0:H], in_=tmp[:].rearrange("p (h c) -> p h c", c=C),
                axis=AX.X, op=ALU.add)
            arl = lb.tile([P, H], F32, tag="arl")
            nc.vector.tensor_tensor(out=tmp[:], in0=v0, in1=adl_s[:],
                                    op=ALU.mult)
            nc.vector.tensor_reduce(
                out=arl[:], in_=tmp[:].rearrange("p (h c) -> p h c", c=C),
                axis=AX.X, op=ALU.add)
            lgp = lb.tile([P, (R + 1) * H], F32, tag="lgp")
            nc.vector.tensor_tensor(
                out=lgp[:].rearrange("p (k h) -> p k h", h=H),
                in0=alp[:].rearrange("p (k h) -> p k h", h=H),
                in1=arl[:, None, :].to_broadcast([P, R + 1, H]),
                op=ALU.add)
            l2p = lb.tile([P, (R + 1) * H], F32, tag="l2p")
            nc.vector.tensor_scalar_mul(out=l2p[:], in0=lgp[:], scalar1=0.2)
            nc.vector.tensor_tensor(out=lgp[:], in0=lgp[:], in1=l2p[:],
                                    op=ALU.max)
            # mask: host-computed bin-occupancy, replicated over heads
            maskp = lb.tile([P, (R + 1) * H], F32, tag="maskp")
            nc.vector.memset(maskp[:, 0:H], 1.0)
            nc.vector.tensor_copy(
                out=maskp[:, H:(R + 1) * H].rearrange("p (r h) -> p r h", h=H),
                in_=maskq_s[:, t * R:(t + 1) * R][:, :, None].to_broadcast(
                    [P, R, H]))
            lm = lb.tile([P, (R + 1) * H], F32, tag="lm")
            nc.vector.tensor_tensor(out=lm[:], in0=lgp[:], in1=maskp[:],
                                    op=ALU.mult)
            mneg = lb.tile([P, (R + 1) * H], F32, tag="mneg")
            nc.vector.tensor_scalar(out=mneg[:], in0=maskp[:], scalar1=1.0,
                                    scalar2=-NEGM, op0=ALU.subtract,
                                    op1=ALU.mult)
            nc.vector.tensor_add(out=lm[:], in0=lm[:], in1=mneg[:])
            ep = lb.tile([P, (R + 1) * H], F32, tag="ep")
            nc.scalar.activation(out=ep[:], in_=lm[:], func=AF.Exp)
            dl = lb.tile([P, H], F32, tag="dl")
            nc.vector.tensor_reduce(
                out=dl[:], in_=ep[:].rearrange("p (k h) -> p h k", h=H),
                axis=AX.X, op=ALU.add)
            rl = lb.tile([P, H], F32, tag="rl")
            nc.vector.reciprocal(out=rl[:], in_=dl[:])
            wga = lb.tile([P, (R + 1) * H], F32, tag="wga")
            nc.vector.tensor_tensor(
                out=wga[:].rearrange("p (k h) -> p k h", h=H),
                in0=ep[:].rearrange("p (k h) -> p k h", h=H),
                in1=rl[:, None, :].to_broadcast([P, R + 1, H]),
                op=ALU.mult)
            # v_r = g_r @ W_cross (reusing gT), then per-head weighted sum
            acc = lb.tile([P, D], F32, tag="acc")
            nc.vector.tensor_tensor(
                out=acc[:].rearrange("p (h c) -> p h c", c=C),
                in0=v0.rearrange("p (h c) -> p h c", c=C),
                in1=wga[:, 0:H][:, :, None].to_broadcast([P, H, C]),
                op=ALU.mult)
            t2 = lb.tile([P, D], F32, tag="t2")
            for r in range(R):
                vv = psL.tile([P, D], F32, tag="vv")
                nc.tensor.matmul(out=vv[:], lhsT=gT[:, r, :], rhs=wcross_s[:],
                                 start=True, stop=True)
                nc.vector.tensor_tensor(
                    out=t2[:].rearrange("p (h c) -> p h c", c=C),
                    in0=vv[:].rearrange("p (h c) -> p h c", c=C),
                    in1=wga[:, (r + 1) * H:(r + 2) * H][:, :, None].to_broadcast(
                        [P, H, C]),
                    op=ALU.mult)
                nc.vector.tensor_add(out=acc[:], in0=acc[:], in1=t2[:])
            nc.vector.tensor_add(out=acc[:], in0=acc[:], in1=bl_s[:])
            go = lb.tile([P, D], F32, tag="go")
            nc.scalar.activation(out=go[:], in_=acc[:], func=AF.Gelu)
            nc.vector.tensor_add(out=go[:], in0=go[:],
                                 in1=xsh_all[:, t * D:(t + 1) * D])
            gob = lb.tile([P, D], BF16, tag="gob")
            nc.vector.tensor_copy(out=gob[:], in_=go[:])
            nc.sync.dma_start(out=out[t * P:(t + 1) * P, :], in_=gob[:])
    return nc


def _prep(x_inp, edge_index, edge_type, W_self, W_word, att_src_word,
          att_dst_word, bias_word, W_cross, att_src_lang, att_dst_lang,
          bias_lang):
    xpad = np.zeros((NPAD, D), np.float32)
    xpad[:N] = x_inp.astype(np.float32)
    src_all = edge_index[0].astype(np.int64)
    dst_all = edge_index[1].astype(np.int64)
    et_all = edge_type.astype(np.int64)

    # shared params
    Wcat = np.zeros((D, R * FD), np.float32)
    Vcat = np.zeros((D, RH), np.float32)
    for r in range(R):
        Wr = W_word[r].astype(np.float32)               # [D, D]
        u = np.einsum('dhc,hc->dh', Wr.reshape(D, H, C),
                      att_src_word[r].astype(np.float32))
        v = np.einsum('dhc,hc->dh', Wr.reshape(D, H, C),
                      att_dst_word[r].astype(np.float32))
        Wcat[:, r * FD:r * FD + D] = Wr
        Wcat[:, r * FD + D:(r + 1) * FD] = u
        Vcat[:, r * H:(r + 1) * H] = v
    Wown = np.concatenate([Vcat, W_self.astype(np.float32)], axis=1)
    Wc = W_cross.astype(np.float32)
    asl = att_src_lang.astype(np.float32)               # [H, C]
    cwm = np.einsum('dhc,hc->dh', Wc.reshape(D, H, C), asl)  # [D, H]
    prow = np.concatenate([
        asl.reshape(D), att_dst_lang.astype(np.float32).reshape(D),
        bias_lang.astype(np.float32).reshape(D),
        bias_word.astype(np.float32).reshape(R * D)]).reshape(1, 8 * D)
    params = {
        "wcat": Wcat.astype(ml_dtypes.bfloat16),
        "wown": Wown.astype(ml_dtypes.bfloat16),
        "wcross": Wc.astype(ml_dtypes.bfloat16),
        "cw": cwm.astype(ml_dtypes.bfloat16),
        "prow": prow,
        "iota_f": np.tile(np.arange(P, dtype=np.float32)[None, :], (P, 1)).astype(ml_dtypes.bfloat16),
        "ident_b": np.eye(P, dtype=np.float32).astype(ml_dtypes.bfloat16),
    }

    # per-core edge binning (dst ownership, global src ids)
    core_of = dst_all // S
    percore = []
    cnts = np.zeros((M, T, R), np.int64)
    for m in range(M):
        sel = core_of == m
        srcm, dstm, etm = src_all[sel], dst_all[sel], et_all[sel]
        dst_l = dstm - m * S
        t_loc = dst_l // P
        order = np.lexsort((dst_l % P, etm, t_loc))
        src_g, dst_l, etm, t_loc = (srcm[order], dst_l[order], etm[order],
                                    t_loc[order])
        cnts[m] = np.bincount(t_loc * R + etm, minlength=T * R).reshape(T, R)
        percore.append((src_g, dst_l, etm, t_loc))

    K = np.maximum(1, -(-cnts.max(axis=0) // P))        # [T, R] chunk counts
    TOTC = int(K.sum())
    coff = np.zeros((T, R), np.int64)                    # chunk offsets
    coff.flat[1:] = np.cumsum(K.flat)[:-1]

    in_maps = []
    for m in range(M):
        src_g, dst_l, etm, t_loc = percore[m]
        sg = np.zeros(TOTC * P, np.int64)
        di = np.full(TOTC * P, 200, np.int64)
        eoff = np.zeros((T, R), np.int64)
        eoff.flat[1:] = np.cumsum(cnts[m].flat)[:-1]
        for t in range(T):
            for r in range(R):
                n_e = cnts[m, t, r]
                if n_e == 0:
                    continue
                o = eoff[t, r]
                slot = coff[t, r] * P + np.arange(n_e)
                sg[slot] = src_g[o:o + n_e]
                di[slot] = dst_l[o:o + n_e] % P
        occ = np.zeros((S, R), np.float32)               # per-node occupancy
        occ[dst_l, etm] = 1.0
        maskm = np.ascontiguousarray(
            occ.reshape(T, P, R).transpose(1, 0, 2).reshape(P, T * R))
        in_maps.append({
            "x_shard": xpad[m * S:(m + 1) * S].astype(ml_dtypes.bfloat16),
            "eidx": np.ascontiguousarray(
                (sg | (di << 16)).astype(np.int32).reshape(TOTC, P).T),
            "maskq": maskm.astype(ml_dtypes.bfloat16),
            **params,
        })
    return K.tolist(), TOTC, in_maps


def kernel(x_inp, node_type, edge_index, edge_type, W_self, W_word,
           att_src_word, att_dst_word, bias_word, W_cross,
           att_src_lang, att_dst_lang, bias_lang):
    global LAST_RESULTS
    K, TOTC, in_maps = _prep(
        np.asarray(x_inp), np.asarray(edge_index), np.asarray(edge_type),
        np.asarray(W_self), np.asarray(W_word), np.asarray(att_src_word),
        np.asarray(att_dst_word), np.asarray(bias_word), np.asarray(W_cross),
        np.asarray(att_src_lang), np.asarray(att_dst_lang),
        np.asarray(bias_lang))
    nc = _build(K, TOTC)
    _split_multiwaits(nc)
    if NSWQ > 1:
        _spread_indirect_queues(nc, NSWQ)
    global LAST_NC, LAST_INMAPS
    LAST_NC, LAST_INMAPS = nc, in_maps
    res = run_bass_kernel_spmd(nc, in_maps, list(range(M)),
                               trace=bool(os.environ.get("BASS_TRACE")))
    LAST_RESULTS = res
    out = np.concatenate([res.results[m]["out"].astype(np.float32)
                          for m in range(M)], axis=0)
    return out[:N]
